# revision 1
# baseline (speedup 1.0000x reference)
"""LinearAttention Trainium2 kernel: data-parallel over batch on 8 NeuronCores.

Reference computation per batch b (C=256 channels, L=4096 seq, H=8 heads, D=64):
  qkv = w_qkv @ x[b]                    # (1536, L)
  q, k, v = split(qkv)                  # each (512, L), rows = (head, dim)
  k = softmax(k, axis=L)
  ctx[h] = k[h] @ v[h].T                # (64, 64)
  out[h] = ctx[h].T @ q[h]              # (64, L)
  y[b] = w_out @ concat(out) + b_out    # (256, L)

Key algebraic restructure: out = ctx.T @ (w_q x) and y = w_out out + b, so
  y = (sum_h w_out_h ctx_h^T w_q_h) x + b = F x + b,    F: (256, 256)
which replaces the Q projection (512x256xL), per-head attention-out
(2x 64x64xL), and output projection (256x512xL) GEMMs with a single
256x256xL GEMM plus a tiny per-batch F build (512x256x(128+256) MACs).
FLOPs per batch drop 1.6x; PE cycles ~164k -> ~102k.

Per-core design (2 batches/core):
  - K^T, V^T computed with L on partitions (lhsT = x chunk, rhs = w^T) so the
    context matmul contracts over L on the TensorEngine.
  - ctx[d,e] accumulated per head-pair with a ones column appended to the V
    rhs: column 128 of the PSUM tile is the softmax denominator sum_l exp(k),
    making the denominator free (129-col matmuls instead of a separate
    ones-lhsT matmul chain + PE transposes).
  - ctx is masked to its two 64x64 diagonal (same-head) blocks and scaled by
    1/den on the PSUM->SBUF copy (per-partition ACT scale), cross-head
    quadrants stay zero in a persistent SBUF tile.
  - F build: H[e,c] = sum_d ctx_norm[d,e] wq[d,c] (lhsT=ctx, rhs=natural-
    layout wq), then F^T[c,o] = sum_he H[he,c] wo^T[he,o] (lhsT=H, rhs=wo^T)
    which lands F already transposed for the final GEMM - no PE transposes.
  - final: y = F^T.T @ x + b, contracting C=256 in 2 chunks of 128.
  - exp() applied unshifted (inputs are N(0,1)-scaled; max |k| ~ 5, safe).
  - all TensorE compute in bf16 (f32 PSUM accumulation); output stored bf16
    and upcast on host.

Schedule/engine choices (from CoreSim timeline analysis):
  - x loads hoisted to rep start, chunked 4x1024 per 128-channel group, on
    the SP queue ahead of all output DMAs; weight loads on the DVE queue so
    the PE's first matmul starts ~3us in instead of ~12us.
  - ctx matmuls issued one lp behind the K/V projections (software pipeline)
    so the PE never waits on the just-produced exp(k)/v tiles.
  - separate PSUM pools for the lp loop vs the tail (H/F/final), so the next
    batch's projections don't queue behind the previous batch's tail in ring
    order; ctx accumulators pair-packed (PSUM banks: 2 ctx + 3 proj +
    3 tail = 8) with a single start per bank.
  - PSUM->SBUF traffic split across engines (Pool cannot read PSUM): exp
    on ACT, V^T/H/F/den on DVE, final bias-copies alternating ACT/DVE,
    ctx-mask quadrant zeroing on Pool.
"""

import numpy as np

B, C, L = 16, 256, 4096
HID = 512
N_CORES = 8
NB = B // N_CORES  # batches per core
CC = C // 128  # contraction chunks for 256-channel GEMMs (2)
LP = L // 128  # l-tiles with l on partitions (32)
LT = L // 512  # l-tiles of 512 for moving-dim matmuls (8)
PR = HID // 128  # head-pairs (4): each 128-wide chunk = 2 heads of 64
XCH = 4  # x DMA chunks per (batch, cc): 4 x 1024 columns

_CACHE = {}


def _build(reps=1, timing=False):
    from concourse import bacc, mybir, tile
    import concourse.bass as bass

    bf16 = mybir.dt.bfloat16
    f32 = mybir.dt.float32
    Exp = mybir.ActivationFunctionType.Exp
    Copy = mybir.ActivationFunctionType.Copy
    Ident = mybir.ActivationFunctionType.Identity

    nc = bacc.Bacc(
        "TRN2",
        target_bir_lowering=False,
        debug=False,
        enable_asserts=False,
        num_devices=N_CORES,
    )

    # Timing builds read x from Internal (uninitialized) DRAM: identical DMA
    # traffic, but the per-call 32MB host->device upload disappears, which
    # would otherwise dominate the wall-clock measurement under axon.
    x_d = nc.dram_tensor(
        "x", [NB, CC, 128, L], bf16, kind="Internal" if timing else "ExternalInput"
    )
    # weights packed host-side into partition-major blobs so each loads
    # with a single DMA per contraction chunk: wkv = [wk^T | wv^T], wqo =
    # [wq_nat | wo^T]
    wkv_d = nc.dram_tensor("wkv", [128, CC, 2, HID], bf16, kind="ExternalInput")
    wqo_d = nc.dram_tensor("wqo", [128, 2, PR, C], bf16, kind="ExternalInput")
    bb_d = nc.dram_tensor("bb", [128, 2], f32, kind="ExternalInput")
    # Timing builds: per-rep disjoint DRAM slices (so repeated work cannot be
    # dead-store-eliminated) in an Internal scratch tensor, with only a tiny
    # ExternalOutput — the axon client fetches ExternalOutputs at ~4GB/s per
    # call, which would otherwise swamp the measurement.
    if timing:
        out_d = nc.dram_tensor("scratch", [reps * NB, 2, 128, L], bf16, kind="Internal")
        chk_d = nc.dram_tensor("chk", [128, reps], bf16, kind="ExternalOutput")
    else:
        out_d = nc.dram_tensor("out", [NB, 2, 128, L], bf16, kind="ExternalOutput")
        chk_d = None

    with tile.TileContext(nc) as tc:
        with (
            tc.tile_pool(name="const", bufs=1) as const,
            tc.tile_pool(name="xp", bufs=2) as xp,
            tc.tile_pool(name="small", bufs=2) as small,
            tc.tile_pool(name="ostp", bufs=3) as ostp,
            tc.tile_pool(name="ps_proj", bufs=3, space="PSUM") as ps_proj,
            tc.tile_pool(name="ps_tail", bufs=3, space="PSUM") as ps_tail,
            tc.tile_pool(name="ps_ctx", bufs=2, space="PSUM") as ps_ctx,
        ):
            wkv = const.tile([128, CC, 2, HID], bf16)
            wqo = const.tile([128, 2, PR, C], bf16)
            bb = const.tile([128, 2], f32)
            ctxs = const.tile([128, PR, 128], bf16)
            # expkt / vta are per-batch streams but live at fixed addresses;
            # vta carries a ones column (idx 128) per (lp, pr) block so the
            # ctx matmul's column 128 accumulates the softmax denominator.
            expkt = const.tile([128, LP, HID], bf16)
            vta = const.tile([128, LP, PR, 129], bf16)

            # the two wkv chunks load on different queues in parallel so the
            # first psk accumulation pair isn't stalled on chunk 1.
            nc.scalar.dma_start(wkv[:, 0], wkv_d[:, 0])
            nc.sync.dma_start(wkv[:, 1], wkv_d[:, 1])
            nc.scalar.dma_start(wqo[:], wqo_d[:])
            nc.scalar.dma_start(bb[:], bb_d[:])
            wqn, wo = wqo[:, 0], wqo[:, 1]
            for lp in range(LP):
                for pr in range(PR):
                    nc.gpsimd.memset(vta[:, lp, pr, 128:129], 1.0)

            XW = L // XCH
            for rep in range(reps):
                if timing:
                    # read back one column of the scratch output so its
                    # stores are observably live, and expose it.
                    chk = small.tile([128, 1], bf16, tag="chk")
                    nc.sync.dma_start(chk[:], out_d[rep * NB, 0, :, 0:1])
                    nc.sync.dma_start(chk_d[:, rep : rep + 1], chk[:])
                # hoist both batches' input loads ahead of all output DMAs
                # on the SP queue (in-order dispatch): interleaved chunks so
                # the first projections can start after ~2 chunks.
                xts = []
                for bi in range(NB):
                    xts.append(
                        xp.tile([128, CC, L], bf16, tag="xt", name=f"xt_{rep}_{bi}")
                    )
                # x loads go on the Pool (SWDGE) queue: decoupled from the SP
                # out-DMA queue so the next rep's loads never wait on this
                # rep's output drain, and Pool is otherwise idle. The first
                # chunks are small so the first projections start sooner.
                xbounds = [0, 512, 1024, 2048, L]
                for bi in range(NB):
                    for xc in range(len(xbounds) - 1):
                        for cc in range(CC):
                            nc.gpsimd.dma_start(
                                xts[bi][:, cc, xbounds[xc] : xbounds[xc + 1]],
                                x_d[bi, cc, :, xbounds[xc] : xbounds[xc + 1]],
                            )

                def lp_phase(bi):
                    xt = xts[bi]
                    # K^T / V^T projections fused with the ctx+den
                    # accumulation, ctx one lp behind the projections:
                    # ctx2[g][d, j, e] = sum_l exp(k[d,l]) v[e,l] for the two
                    # head-pairs j; col 128 = den[d].
                    # pair-packed: two [128,2,129] tiles, each one PSUM bank.
                    # start=True only on the bank's first matmul: start marks
                    # the whole 2KB zero region pending-zero, so the second
                    # slice's first write is zeroed without its own start
                    # (a second start would re-mark the region and wipe the
                    # first slice's lp=0 contribution).
                    ctx2 = [
                        ps_ctx.tile(
                            [128, 2, 129], f32, tag="ctx", name=f"ctx_{rep}_{bi}_{g}"
                        )
                        for g in range(2)
                    ]
                    for lp in range(LP + 1):
                        if lp < LP:
                            psk = ps_proj.tile([128, HID], f32, tag="mm")
                            psv = ps_proj.tile([128, PR, 128], f32, tag="mm")
                            for cc in range(CC):
                                nc.tensor.matmul(
                                    psk[:],
                                    xt[:, cc, lp * 128 : (lp + 1) * 128],
                                    wkv[:, cc, 0, :],
                                    start=(cc == 0),
                                    stop=(cc == CC - 1),
                                )
                            for cc in range(CC):
                                nc.tensor.matmul(
                                    psv[:],
                                    xt[:, cc, lp * 128 : (lp + 1) * 128],
                                    wkv[:, cc, 1, :],
                                    start=(cc == 0),
                                    stop=(cc == CC - 1),
                                )
                            nc.scalar.activation(expkt[:, lp, :], psk[:], Exp)
                            nc.vector.tensor_copy(vta[:, lp, :, 0:128], psv[:])
                        if lp > 0:
                            lq = lp - 1
                            for pr in range(PR):
                                nc.tensor.matmul(
                                    ctx2[pr // 2][:, pr % 2, :],
                                    expkt[:, lq, pr * 128 : (pr + 1) * 128],
                                    vta[:, lq, pr, :],
                                    start=(lq == 0 and pr % 2 == 0),
                                    stop=(lq == LP - 1 and pr % 2 == 1),
                                )
                    return ctx2

                def hf_phase(bi, ctx2):
                    inv_den = small.tile([128, PR], f32, tag="invden")
                    for pr in range(PR):
                        nc.vector.reciprocal(
                            inv_den[:, pr : pr + 1],
                            ctx2[pr // 2][:, pr % 2, 128:129],
                        )
                    # 1/den-scaled full-tile copies split across ACT and DVE,
                    # then cross-head 64x64 quadrants zeroed on Pool.
                    for pr in range(PR):
                        if pr < 2:
                            nc.scalar.activation(
                                ctxs[:, pr, :],
                                ctx2[pr // 2][:, pr % 2, 0:128],
                                Copy,
                                scale=inv_den[:, pr : pr + 1],
                            )
                        else:
                            nc.vector.tensor_scalar_mul(
                                ctxs[:, pr, :],
                                ctx2[pr // 2][:, pr % 2, 0:128],
                                inv_den[:, pr : pr + 1],
                            )
                        nc.gpsimd.memset(ctxs[0:64, pr, 64:128], 0.0)
                        nc.gpsimd.memset(ctxs[64:128, pr, 0:64], 0.0)

                    # H[e, c] = sum_d ctx_norm[d, e] wq[d, c]
                    hs = small.tile([128, PR, C], bf16, tag="hs")
                    for pr in range(PR):
                        hp = ps_tail.tile([128, C], f32, tag="mm")
                        nc.tensor.matmul(
                            hp[:], ctxs[:, pr, :], wqn[:, pr, :], start=True, stop=True
                        )
                        nc.vector.tensor_copy(hs[:, pr, :], hp[:])
                    # F^T[c, o] = sum_he H[he, c] wo^T[he, o]
                    fts = small.tile([128, CC, C], bf16, tag="fts")
                    for cc in range(CC):
                        ftp = ps_tail.tile([128, C], f32, tag="mm")
                        for pr in range(PR):
                            nc.tensor.matmul(
                                ftp[:],
                                hs[:, pr, cc * 128 : (cc + 1) * 128],
                                wo[:, pr, :],
                                start=(pr == 0),
                                stop=(pr == PR - 1),
                            )
                        nc.vector.tensor_copy(fts[:, cc, :], ftp[:])
                    return fts

                def fx_phase(bi, fts, lo=0, hi=LT // 2):
                    xt = xts[bi]
                    # y = F^T.T @ x + b, streamed over L in chunks of 512;
                    # output staged in 1024-col pairs so the SP sequencer
                    # dispatches 8 out-DMAs per batch instead of 32.
                    for ltp in range(lo, hi):
                        ostg = ostp.tile([128, 2, 1024], bf16, tag="ostg")
                        for lth in range(2):
                            lt = 2 * ltp + lth
                            for oc2 in range(2):
                                psf = ps_tail.tile([128, 512], f32, tag="mm")
                                for cc in range(CC):
                                    nc.tensor.matmul(
                                        psf[:],
                                        fts[:, cc, oc2 * 128 : (oc2 + 1) * 128],
                                        xt[:, cc, lt * 512 : (lt + 1) * 512],
                                        start=(cc == 0),
                                        stop=(cc == CC - 1),
                                    )
                                dst = ostg[:, oc2, lth * 512 : (lth + 1) * 512]
                                if oc2 == 0:
                                    nc.scalar.activation(
                                        dst, psf[:], Ident, bias=bb[:, oc2 : oc2 + 1]
                                    )
                                else:
                                    nc.vector.tensor_scalar_add(
                                        dst, psf[:], bb[:, oc2 : oc2 + 1]
                                    )
                        obi = (rep * NB + bi) if timing else bi
                        ocol = ltp * 1024
                        # last batch's final chunks go out as 512-col DMAs
                        # (the very last on the ACT queue) so the
                        # post-compute drain is one short transfer per queue.
                        last = bi == NB - 1 and ltp == LT // 2 - 1
                        for oc2 in range(2):
                            if last:
                                nc.sync.dma_start(
                                    out_d[obi, oc2, :, ocol : ocol + 512],
                                    ostg[:, oc2, 0:512],
                                )
                                (nc.scalar if oc2 else nc.sync).dma_start(
                                    out_d[obi, oc2, :, ocol + 512 : ocol + 1024],
                                    ostg[:, oc2, 512:1024],
                                )
                            else:
                                nc.sync.dma_start(
                                    out_d[obi, oc2, :, ocol : ocol + 1024],
                                    ostg[:, oc2, :],
                                )

                # interleave batch phases so the last batch's den/mask/H
                # latency chain hides under the previous batch's final GEMM:
                # lp(0), hf(0), lp(1), fx(0), hf(1), fx(1).
                prev = None
                for bi in range(NB):
                    ctx2 = lp_phase(bi)
                    if bi + 1 < NB:
                        prev = (bi, hf_phase(bi, ctx2))
                    else:
                        if prev is not None:
                            fx_phase(*prev, 0, 2)
                        fts = hf_phase(bi, ctx2)
                        if prev is not None:
                            fx_phase(*prev, 2, LT // 2)
                        fx_phase(bi, fts)

    nc.compile()
    return nc


def _get_nc():
    if "nc" not in _CACHE:
        _CACHE["nc"] = _build()
    return _CACHE["nc"]


def _prep_in_maps(x, w_qkv, w_out, b_out):
    import ml_dtypes

    bf16 = ml_dtypes.bfloat16
    wqn = w_qkv[0:512].reshape(PR, 128, C)
    wk_t = np.ascontiguousarray(w_qkv[512:1024].T).reshape(CC, 128, HID)
    wv_t = np.ascontiguousarray(w_qkv[1024:1536].T).reshape(CC, 128, HID)
    wo_t = np.ascontiguousarray(w_out.T).reshape(PR, 128, C)
    wkv = np.ascontiguousarray(
        np.stack([wk_t, wv_t], axis=1).transpose(2, 0, 1, 3)
    ).astype(bf16)
    wqo = np.ascontiguousarray(
        np.stack([wqn.transpose(1, 0, 2), wo_t.transpose(1, 0, 2)], axis=1)
    ).astype(bf16)
    bb = np.ascontiguousarray(b_out.reshape(2, 128).T).astype(np.float32)
    in_maps = []
    for c in range(N_CORES):
        xs = x[c * NB : (c + 1) * NB].reshape(NB, CC, 128, L).astype(bf16)
        in_maps.append(
            {
                "x": np.ascontiguousarray(xs),
                "wkv": wkv,
                "wqo": wqo,
                "bb": bb,
            }
        )
    return in_maps


def kernel(x, w_qkv, w_out, b_out):
    from concourse.bass_utils import run_bass_kernel_spmd

    nc = _get_nc()
    in_maps = _prep_in_maps(
        np.asarray(x, dtype=np.float32),
        np.asarray(w_qkv, dtype=np.float32),
        np.asarray(w_out, dtype=np.float32),
        np.asarray(b_out, dtype=np.float32),
    )
    res = run_bass_kernel_spmd(nc, in_maps, core_ids=list(range(N_CORES)))
    out = np.concatenate(
        [
            res.results[c]["out"].astype(np.float32).reshape(NB, C, L)
            for c in range(N_CORES)
        ],
        axis=0,
    )
    return out

